# revision 5
# baseline (speedup 1.0000x reference)
"""MoE top-2 router kernel for 8 Trainium2 NeuronCores.

Strategy (data parallel over tokens, hint-compliant):
  - Flatten tokens: [4,4096,4096] -> [16384,4096]; core i takes tokens
    [i*2048, (i+1)*2048).
  - Host prep per core: transpose shard to [H, T] and split fp32 into
    fp16 hi/lo planes (x = hi + lo, ~2^-22 effective mantissa), stored
    interleaved as xcat [H, 2, T] so each 128-row chunk is ONE 1 MiB DMA.
  - Router weight (replicated): W^T split the same way into
    wcat [H, 16] = [Whi | Wlo].
  - Device: logits^T[e, t] accumulated in PSUM over 32 h-chunks with two
    fp16 matmuls per (chunk, 512-token group):
       mm1: [Whi|Wlo] (stationary [128,16]) x hi -> psum rows 0..15
       mm2:  Whi      (stationary [128,8])  x lo -> psum rows 32..39
    logits = rows0:8 + rows8:16 + rows32:40  (exact fp32 accumulate).
  - Epilogue: PE-transpose logits^T back to [tokens, 8], DVE max/max_index
    (top-8 sorted desc, ties -> lowest index first == jax.lax.top_k),
    weights w1 = 1/(1+exp(l2-l1)), w2 = e*w1 (== renormalized softmax).
"""
import sys
import numpy as np

if "/opt/trn_rl_repo" not in sys.path:
    sys.path.insert(0, "/opt/trn_rl_repo")

B, S, H, E = 4, 4096, 4096, 8
NCORES = 8
T = B * S              # 16384 tokens
TS = T // NCORES       # 2048 tokens / core
KC = H // 128          # 32 contraction chunks
NG = TS // 512         # 4 moving groups of 512
NCHUNK = TS // 128     # 16 token chunks of 128

_nc_cache = {}


def build_nc(repeat=1):
    if repeat in _nc_cache:
        return _nc_cache[repeat]
    import concourse.bacc as bacc
    import concourse.tile as tile
    import concourse.mybir as mybir

    f16, f32, u32 = mybir.dt.float16, mybir.dt.float32, mybir.dt.uint32
    nc = bacc.Bacc("TRN2", target_bir_lowering=False, debug=False)

    xcat = nc.declare_dram_parameter("xcat", [H, 2, TS], f16, isOutput=False)
    wcat = nc.declare_dram_parameter("wcat", [H, 2 * E], f16, isOutput=False)
    out_logits = nc.declare_dram_parameter("logits", [TS, E], f32, isOutput=True)
    out_w = nc.declare_dram_parameter("weights", [TS, 2], f32, isOutput=True)
    out_i = nc.declare_dram_parameter("indices", [TS, 2], u32, isOutput=True)
    ident = nc.inline_tensor(np.eye(2 * E, dtype=np.float32), name="ident16")

    with tile.TileContext(nc) as tc:
        with (
            tc.tile_pool(name="consts", bufs=1) as cpool,
            tc.tile_pool(name="xp", bufs=6) as xpool,
            tc.tile_pool(name="ep", bufs=2) as ep,
            tc.tile_pool(name="ps", bufs=1, space="PSUM") as pspool,
            tc.tile_pool(name="pst", bufs=2, space="PSUM") as pstpool,
        ):
            wsb = cpool.tile([128, KC, 2 * E], f16)
            nc.sync.dma_start(
                wsb[:], wcat.ap().rearrange("(k p) e -> p k e", p=128)
            )
            id_t = cpool.tile([2 * E, 2 * E], f32)
            nc.sync.dma_start(id_t[:], ident.ap())

            for _r in range(repeat):
                psums = [
                    pspool.tile([2 * E, 512], f32, tag=f"ps{g}", name=f"ps{g}")
                    for g in range(NG)
                ]
                for k in range(KC):
                    xt = xpool.tile([128, 2, TS], f16, tag="xt")
                    nc.sync.dma_start(xt[:], xcat.ap()[k * 128:(k + 1) * 128])
                    for g in range(NG):
                        # rows 0..7 += x@Whi, rows 8..15 += x@Wlo (hi then lo
                        # moving planes; same stationary) -> after both passes
                        # over all k, rows0:8 + rows8:16 == exact x @ W^T.
                        nc.tensor.matmul(
                            psums[g][:, :],
                            wsb[:, k, :],
                            xt[:, 0, g * 512:(g + 1) * 512],
                            start=(k == 0),
                            stop=False,
                        )
                        nc.tensor.matmul(
                            psums[g][:, :],
                            wsb[:, k, :],
                            xt[:, 1, g * 512:(g + 1) * 512],
                            start=False,
                            stop=(k == KC - 1),
                        )

                logits_sb = ep.tile([2 * E, TS], f32, tag="logits_sb")
                for g in range(NG):
                    nc.scalar.copy(
                        logits_sb[:, g * 512:(g + 1) * 512], psums[g][:, :]
                    )

                logitsT = ep.tile([128, NCHUNK, E], f32, tag="logitsT")
                for c in range(NCHUNK):
                    pt = pstpool.tile([128, 2 * E], f32, tag="pt")
                    nc.tensor.transpose(
                        pt[:], logits_sb[:, c * 128:(c + 1) * 128], id_t[:]
                    )
                    nc.scalar.copy(logitsT[:, c, :], pt[:, 0:E])
                    nc.vector.tensor_add(
                        logitsT[:, c, :], logitsT[:, c, :], pt[:, E:2 * E]
                    )

                m8 = ep.tile([128, NCHUNK, E], f32, tag="m8")
                i8 = ep.tile([128, NCHUNK, E], u32, tag="i8")
                for c in range(NCHUNK):
                    nc.vector.max(m8[:, c, :], logitsT[:, c, :])
                    nc.vector.max_index(i8[:, c, :], m8[:, c, :], logitsT[:, c, :])

                d = ep.tile([128, NCHUNK, 1], f32, tag="d")
                nc.vector.tensor_sub(d[:], m8[:, :, 1:2], m8[:, :, 0:1])
                e_t = ep.tile([128, NCHUNK, 1], f32, tag="e_t")
                nc.scalar.activation(
                    e_t[:], d[:], mybir.ActivationFunctionType.Exp
                )
                s_t = ep.tile([128, NCHUNK, 1], f32, tag="s_t")
                nc.vector.tensor_scalar_add(s_t[:], e_t[:], 1.0)
                wout = ep.tile([128, NCHUNK, 2], f32, tag="wout")
                nc.vector.reciprocal(wout[:, :, 0:1], s_t[:])
                nc.vector.tensor_mul(wout[:, :, 1:2], e_t[:], wout[:, :, 0:1])

                nc.sync.dma_start(
                    out_logits.ap().rearrange("(c p) e -> p c e", p=128), logitsT[:]
                )
                nc.sync.dma_start(
                    out_w.ap().rearrange("(c p) e -> p c e", p=128), wout[:]
                )
                nc.sync.dma_start(
                    out_i.ap().rearrange("(c p) e -> p c e", p=128), i8[:, :, 0:2]
                )

    nc.compile()
    _nc_cache[repeat] = nc
    return nc


def _split_w(router_weight):
    wT = np.ascontiguousarray(np.asarray(router_weight, dtype=np.float32).T)
    whi = wT.astype(np.float16)
    wlo = (wT - whi.astype(np.float32)).astype(np.float16)
    return np.ascontiguousarray(np.concatenate([whi, wlo], axis=1))


def _prep_core(hs, i, wcat_np):
    xT = hs[i * TS:(i + 1) * TS].T          # [H, TS] strided view
    xcat = np.empty((H, 2, TS), np.float16)
    xhi = xT.astype(np.float16)
    xcat[:, 0, :] = xhi
    xcat[:, 1, :] = (xT - xhi.astype(np.float32)).astype(np.float16)
    return {"xcat": xcat, "wcat": wcat_np}


def make_in_maps(hidden_states, router_weight):
    hs = np.ascontiguousarray(
        np.asarray(hidden_states, dtype=np.float32).reshape(T, H)
    )
    wcat_np = _split_w(router_weight)
    from concurrent.futures import ThreadPoolExecutor
    with ThreadPoolExecutor(NCORES) as ex:
        return list(ex.map(lambda i: _prep_core(hs, i, wcat_np), range(NCORES)))


def _assemble(results):
    logits = np.concatenate([r["logits"] for r in results], axis=0)
    wts = np.concatenate([r["weights"] for r in results], axis=0)
    idx = np.concatenate([r["indices"] for r in results], axis=0)
    return (
        wts.reshape(B, S, 2),
        idx.astype(np.int32).reshape(B, S, 2),
        logits.reshape(B, S, E),
    )


def kernel(hidden_states, router_weight):
    from concourse.bass_utils import run_bass_kernel_spmd
    nc = build_nc(1)
    in_maps = make_in_maps(hidden_states, router_weight)
    res = run_bass_kernel_spmd(nc, in_maps, core_ids=list(range(NCORES)))
    return _assemble(res.results)


# revision 8
# speedup vs baseline: 7.1782x; 7.1782x over previous
"""MoE top-2 router kernel for 8 Trainium2 NeuronCores.

Strategy (data parallel over tokens, hint-compliant):
  - Flatten tokens: [4,4096,4096] -> [16384,4096]; core i takes tokens
    [i*2048, (i+1)*2048).
  - Host prep per core: transpose shard to [H, T] and split fp32 into
    fp16 hi/lo planes (x = hi + lo, ~2^-22 effective mantissa), stored
    interleaved as xcat [H, 2, T] so each 128-row chunk is ONE 1 MiB DMA.
  - Router weight (replicated): W^T split the same way into
    wcat [H, 16] = [Whi | Wlo].
  - Device: logits^T[e, t] accumulated in PSUM over 32 h-chunks with two
    fp16 matmuls per (chunk, 512-token group):
       mm1: [Whi|Wlo] (stationary [128,16]) x hi -> psum rows 0..15
       mm2:  Whi      (stationary [128,8])  x lo -> psum rows 32..39
    logits = rows0:8 + rows8:16 + rows32:40  (exact fp32 accumulate).
  - Epilogue: PE-transpose logits^T back to [tokens, 8], DVE max/max_index
    (top-8 sorted desc, ties -> lowest index first == jax.lax.top_k),
    weights w1 = 1/(1+exp(l2-l1)), w2 = e*w1 (== renormalized softmax).
"""
import sys
import numpy as np

if "/opt/trn_rl_repo" not in sys.path:
    sys.path.insert(0, "/opt/trn_rl_repo")

B, S, H, E = 4, 4096, 4096, 8
NCORES = 8
T = B * S              # 16384 tokens
TS = T // NCORES       # 2048 tokens / core
KC = H // 128          # 32 contraction chunks
NG = TS // 512         # 4 moving groups of 512
NCHUNK = TS // 128     # 16 token chunks of 128

_nc_cache = {}


def build_nc(repeat=1):
    if repeat in _nc_cache:
        return _nc_cache[repeat]
    import concourse.bacc as bacc
    import concourse.tile as tile
    import concourse.mybir as mybir

    f16, f32, u32 = mybir.dt.float16, mybir.dt.float32, mybir.dt.uint32
    nc = bacc.Bacc("TRN2", target_bir_lowering=False, debug=False)

    xcat = nc.declare_dram_parameter("xcat", [H, 2, TS], f16, isOutput=False)
    wcat = nc.declare_dram_parameter("wcat", [H, 2 * E], f16, isOutput=False)
    out_logits = nc.declare_dram_parameter("logits", [TS, E], f32, isOutput=True)
    out_w = nc.declare_dram_parameter("weights", [TS, 2], f32, isOutput=True)
    out_i = nc.declare_dram_parameter("indices", [TS, 2], u32, isOutput=True)
    ident = nc.inline_tensor(np.eye(2 * E, dtype=np.float32), name="ident16")

    with tile.TileContext(nc) as tc:
        with (
            tc.tile_pool(name="consts", bufs=1) as cpool,
            tc.tile_pool(name="xp", bufs=8) as xpool,
            tc.tile_pool(name="ep", bufs=2) as ep,
            tc.tile_pool(name="ps", bufs=1, space="PSUM") as pspool,
            tc.tile_pool(name="pst", bufs=2, space="PSUM") as pstpool,
        ):
            wsb = cpool.tile([128, KC, 2 * E], f16)
            nc.sync.dma_start(
                wsb[:], wcat.ap().rearrange("(k p) e -> p k e", p=128)
            )
            id_t = cpool.tile([2 * E, 2 * E], f32)
            nc.sync.dma_start(id_t[:], ident.ap())

            for _r in range(repeat):
                psums = [
                    pspool.tile([2 * E, 512], f32, tag=f"ps{g}", name=f"ps{g}")
                    for g in range(NG)
                ]
                for k in range(KC):
                    xt = xpool.tile([128, 2, TS], f16, tag="xt")
                    # split across both physical HWDGE rings (SP + ACT)
                    nc.sync.dma_start(
                        xt[:, 0, :], xcat.ap()[k * 128:(k + 1) * 128, 0]
                    )
                    nc.scalar.dma_start(
                        xt[:, 1, :], xcat.ap()[k * 128:(k + 1) * 128, 1]
                    )
                    for g in range(NG):
                        # rows 0..7 += x@Whi, rows 8..15 += x@Wlo (hi then lo
                        # moving planes; same stationary) -> after both passes
                        # over all k, rows0:8 + rows8:16 == exact x @ W^T.
                        nc.tensor.matmul(
                            psums[g][:, :],
                            wsb[:, k, :],
                            xt[:, 0, g * 512:(g + 1) * 512],
                            start=(k == 0),
                            stop=False,
                        )
                        nc.tensor.matmul(
                            psums[g][:, :],
                            wsb[:, k, :],
                            xt[:, 1, g * 512:(g + 1) * 512],
                            start=False,
                            stop=(k == KC - 1),
                        )

                logits_sb = ep.tile([2 * E, TS], f32, tag="logits_sb")
                for g in range(NG):
                    nc.scalar.copy(
                        logits_sb[:, g * 512:(g + 1) * 512], psums[g][:, :]
                    )

                logitsT = ep.tile([128, NCHUNK, E], f32, tag="logitsT")
                for c in range(NCHUNK):
                    pt = pstpool.tile([128, 2 * E], f32, tag="pt")
                    nc.tensor.transpose(
                        pt[:], logits_sb[:, c * 128:(c + 1) * 128], id_t[:]
                    )
                    nc.scalar.copy(logitsT[:, c, :], pt[:, 0:E])
                    nc.vector.tensor_add(
                        logitsT[:, c, :], logitsT[:, c, :], pt[:, E:2 * E]
                    )

                m8 = ep.tile([128, NCHUNK, E], f32, tag="m8")
                i8 = ep.tile([128, NCHUNK, E], u32, tag="i8")
                for c in range(NCHUNK):
                    nc.vector.max(m8[:, c, :], logitsT[:, c, :])
                    nc.vector.max_index(i8[:, c, :], m8[:, c, :], logitsT[:, c, :])

                d = ep.tile([128, NCHUNK, 1], f32, tag="d")
                nc.vector.tensor_sub(d[:], m8[:, :, 1:2], m8[:, :, 0:1])
                e_t = ep.tile([128, NCHUNK, 1], f32, tag="e_t")
                nc.scalar.activation(
                    e_t[:], d[:], mybir.ActivationFunctionType.Exp
                )
                s_t = ep.tile([128, NCHUNK, 1], f32, tag="s_t")
                nc.vector.tensor_scalar_add(s_t[:], e_t[:], 1.0)
                wout = ep.tile([128, NCHUNK, 2], f32, tag="wout")
                nc.vector.reciprocal(wout[:, :, 0:1], s_t[:])
                nc.vector.tensor_mul(wout[:, :, 1:2], e_t[:], wout[:, :, 0:1])

                nc.sync.dma_start(
                    out_logits.ap().rearrange("(c p) e -> p c e", p=128), logitsT[:]
                )
                nc.scalar.dma_start(
                    out_w.ap().rearrange("(c p) e -> p c e", p=128), wout[:]
                )
                nc.scalar.dma_start(
                    out_i.ap().rearrange("(c p) e -> p c e", p=128), i8[:, :, 0:2]
                )

    nc.compile()
    _nc_cache[repeat] = nc
    return nc


def _split_w(router_weight):
    wT = np.ascontiguousarray(np.asarray(router_weight, dtype=np.float32).T)
    whi = wT.astype(np.float16)
    wlo = (wT - whi.astype(np.float32)).astype(np.float16)
    return np.ascontiguousarray(np.concatenate([whi, wlo], axis=1))


def _prep_core(hs, i, wcat_np):
    xT = hs[i * TS:(i + 1) * TS].T          # [H, TS] strided view
    xcat = np.empty((H, 2, TS), np.float16)
    xhi = xT.astype(np.float16)
    xcat[:, 0, :] = xhi
    xcat[:, 1, :] = (xT - xhi.astype(np.float32)).astype(np.float16)
    return {"xcat": xcat, "wcat": wcat_np}


def make_in_maps(hidden_states, router_weight):
    hs = np.ascontiguousarray(
        np.asarray(hidden_states, dtype=np.float32).reshape(T, H)
    )
    wcat_np = _split_w(router_weight)
    from concurrent.futures import ThreadPoolExecutor
    with ThreadPoolExecutor(NCORES) as ex:
        return list(ex.map(lambda i: _prep_core(hs, i, wcat_np), range(NCORES)))


def _assemble(results):
    logits = np.concatenate([r["logits"] for r in results], axis=0)
    wts = np.concatenate([r["weights"] for r in results], axis=0)
    idx = np.concatenate([r["indices"] for r in results], axis=0)
    return (
        wts.reshape(B, S, 2),
        idx.astype(np.int32).reshape(B, S, 2),
        logits.reshape(B, S, E),
    )


def kernel(hidden_states, router_weight):
    from concourse.bass_utils import run_bass_kernel_spmd
    nc = build_nc(1)
    in_maps = make_in_maps(hidden_states, router_weight)
    res = run_bass_kernel_spmd(nc, in_maps, core_ids=list(range(NCORES)))
    return _assemble(res.results)
